# revision 53
# baseline (speedup 1.0000x reference)
"""Multi-head attention block (LN -> QKV -> attention -> out-proj) on 8 TRN2 cores.

Sharding: (batch, head-half). Core i handles batch i//2 and heads
8*(i%2) .. 8*(i%2)+8 for ALL 2048 query rows. Each core computes LN for
its whole batch, Q/K/V for its 8 heads, attention, and a PARTIAL
out-projection (its heads' slice of w_out rows). The host sums the two
partial outputs per batch (+ b_out). This removes the K/V-projection
duplication of a (batch, query-half) split: per-core projection work
drops from 768 to 512 N=512 matmuls.

All 8 cores run ONE graph; per-core inputs differ (x per batch, weight
slices per head-half).

Compute dtype bf16 (PSUM accumulation f32). Host folds ln_gamma and the
softmax scale into w_qkv (exact f32); ln_beta @ w_qkv is asserted zero.

Attention data path (per head pair, per 512-query chunk):
  dots^T = K @ Q^T as two K=64 row-tiled matmuls (tile rows 0/64) into
  ONE [128, 1024] PSUM tile (adjacent banks) -> a single 1024-wide exp
  on ScalarE writes attn^T bf16.
  attn@V uses V_aug as the STATIONARY operand with a 128-COLUMN access
  pattern (cols = [V_h | ones | next head's data / pad]) so the fast
  weight load path (FWL needs a 128x128 stationary) keeps LDWEIGHTS off
  the critical path; PSUM rows 65:128 accumulate junk that is never
  read. Row 64 accumulates the softmax denominator.
  The denominator row is broadcast to 64 partitions with two DVE
  stream_shuffles, reciprocal'd with the fast approx op (~5x cheaper
  than exact; denominators are O(1e2..1e3), far from its edge cases),
  and multiplied into the out rows by one tensor_tensor -> ot tile.
  attn@V for chunk c is emitted interleaved into the dots/exp loop of
  chunk c+1, so the PE never waits on the exp stream.

Partial outputs are written bf16 (halves the 8MB/core output DMA; the
host sums the two per-batch partials in f32 -- the added rounding is
~0.4% RMS against a 2e-2 budget). Steady state runs the PE at ~99%
and ScalarE (exp) at ~97% concurrently; both engines are at their
structural floor (PE: 512 proj + 512 attn@V + 256 dots-pair matmul
slots, all N=512 streams; ScalarE: 256 1024-wide exps).
"""

import sys

sys.path.insert(0, "/opt/trn_rl_repo")

import ml_dtypes
import numpy as np

import concourse.bass as bass
import concourse.tile as tile
from concourse import bacc, mybir
from concourse.bass_utils import run_bass_kernel_spmd
from concourse.masks import make_identity

F32 = mybir.dt.float32
BF16 = mybir.dt.bfloat16
AF = mybir.ActivationFunctionType

B, N, DIM = 4, 2048, 1024
HEADS, DH = 16, 64
INNER = HEADS * DH  # 1024
SCALE = DH ** -0.5
N_CORES = 8
RT = N // 128         # 16 row tiles
KT = DIM // 128       # 8 contraction tiles (qkv proj)
MYH = HEADS // 2      # 8 heads per core
PAIRS = MYH // 2      # 4 head pairs per core
CHS = N // 512        # 4 query chunks of 512
KO = MYH * DH // 128  # 4 contraction tiles (out proj)
VW = MYH * 65 + 63    # v_sb free width per row-tile (65-per-head + pad)
EPS = 1e-5


def _build_graph():
    nc = bacc.Bacc("TRN2", target_bir_lowering=False, debug=False,
                   num_devices=N_CORES)
    x_d = nc.dram_tensor("x", [N, DIM], BF16, kind="ExternalInput").ap()
    # per-core slice: [q 512 | k 512 | v 512] columns of w_qkv
    wqkv_d = nc.dram_tensor("wqkv", [DIM, 3 * 512], BF16,
                            kind="ExternalInput").ap()
    # per-core slice: rows hh*512:(hh+1)*512 of w_out
    wout_d = nc.dram_tensor("wout", [512, DIM], BF16,
                            kind="ExternalInput").ap()
    # bf16 partial outputs: the host sums the two per-batch partials in
    # f32; the extra ~0.4% rounding is far inside the error budget and
    # halves the output DMA (8MB -> 4MB per core)
    out_d = nc.dram_tensor("out", [N, DIM], BF16, kind="ExternalOutput").ap()

    with tile.TileContext(nc) as tc:
        _kernel_body(tc, x_d, wqkv_d, wout_d, out_d)
    nc.compile()
    return nc


def _kernel_body(tc, x_d, wqkv_d, wout_d, out_d):
    nc = tc.nc
    from contextlib import ExitStack

    with ExitStack() as outer:
        const_pool = outer.enter_context(tc.tile_pool(name="const", bufs=1))
        persist = outer.enter_context(tc.tile_pool(name="persist", bufs=1))
        # NOTE: denser PE packing (3 dots buffers + 2-t grouping) was
        # tried and REGRESSED: the sustained matmul density trips the
        # chip's activity throttle / HAM oscillations. The ~107ns
        # contraction-row-switch stalls per t are effectively free
        # power headroom. Keep 2+2+2+2 PSUM pools and per-t interleave.
        pools = {}
        pools["psm"] = outer.enter_context(
            tc.tile_pool(name="psm", bufs=2, space=bass.MemorySpace.PSUM))
        pools["psd"] = outer.enter_context(
            tc.tile_pool(name="psd", bufs=2, space=bass.MemorySpace.PSUM))

        ident = const_pool.tile([128, 128], BF16, tag="ident")
        make_identity(nc, ident[:])
        eps_t = const_pool.tile([128, 1], F32, tag="eps")
        nc.gpsimd.memset(eps_t[:], EPS)
        # preload the Sqrt and Exp activation tables with dummy ops while
        # the first x DMA is still in flight -- the first real rstd/exp
        # otherwise pays a ~1.3us ACT_TABLE_LOAD on the critical path
        warm_t = const_pool.tile([1, 1], F32, tag="actwarm")
        nc.scalar.activation(out=warm_t[:], in_=eps_t[0:1, :], func=AF.Sqrt)
        nc.scalar.activation(out=warm_t[:], in_=eps_t[0:1, :], func=AF.Exp)


        # persistent through the whole kernel
        xnt = persist.tile([128, KT, N], BF16, tag="xnt")      # [dim, kt, row]
        v_sb = persist.tile([128, RT, VW], BF16, tag="v")      # [kv,rt,h*65+d]

        # per-pair projection staging (lives through the pipeline)
        wqkp = outer.enter_context(tc.tile_pool(name="wqkp", bufs=2))
        qtp_pool = outer.enter_context(tc.tile_pool(name="qtp", bufs=2))
        ktp_pool = outer.enter_context(tc.tile_pool(name="ktp", bufs=2))
        ap_pool = outer.enter_context(tc.tile_pool(name="attn", bufs=2))

        def stage_weights(p):
            wqk = wqkp.tile([128, KT, 256], BF16, tag="wqk")
            for k in range(KT):
                nc.sync.dma_start(
                    wqk[:, k, 0:128],
                    wqkv_d[k * 128:(k + 1) * 128, p * 128:(p + 1) * 128])
                nc.sync.dma_start(
                    wqk[:, k, 128:256],
                    wqkv_d[k * 128:(k + 1) * 128,
                           512 + p * 128:512 + (p + 1) * 128])
            return wqk

        def proj_q_chunk(wqk, qt_p, ch):
            ps = pools["psm"].tile([128, 512], F32, tag="mm")
            for k in range(KT):
                nc.tensor.matmul(
                    ps[:], wqk[:, k, 0:128],
                    xnt[:, k, ch * 512:(ch + 1) * 512],
                    start=(k == 0), stop=(k == KT - 1))
            nc.vector.tensor_copy(out=qt_p[:, ch * 512:(ch + 1) * 512],
                                  in_=ps[:])

        def proj_k_chunk(wqk, kt_p, ch):
            ps = pools["psm"].tile([128, 512], F32, tag="mm")
            for k in range(KT):
                nc.tensor.matmul(
                    ps[:], wqk[:, k, 128:256],
                    xnt[:, k, ch * 512:(ch + 1) * 512],
                    start=(k == 0), stop=(k == KT - 1))
            nc.vector.tensor_copy(out=kt_p[:, ch * 512:(ch + 1) * 512],
                                  in_=ps[:])

        def alloc_pair():
            qt_p = qtp_pool.tile([128, N], BF16, tag="qt")
            kt_p = ktp_pool.tile([128, N], BF16, tag="kt")
            return qt_p, kt_p

        def dots_exp(qt_p, kt_p, at, ch, t):
            # both heads of the pair into one [128,1024] psum (2 banks),
            # K=64 row tiles 0/64 run concurrently in the PE array
            ps = pools["psd"].tile([128, 1024], F32, tag="dots")
            for hi in range(2):
                nc.tensor.matmul(
                    ps[:, hi * 512:(hi + 1) * 512],
                    kt_p[hi * DH:(hi + 1) * DH, t * 128:(t + 1) * 128],
                    qt_p[hi * DH:(hi + 1) * DH, ch * 512:(ch + 1) * 512],
                    start=True, stop=True)
            nc.scalar.activation(out=at[:, t, :, :], in_=ps[:], func=AF.Exp)

        # ---- phase 1: LayerNorm + transpose into xnt; V projection ----
        # x row-tiles 0..3 are DMA'd before any weight staging so LN and
        # the PE transposes start within ~2us of kernel entry.
        at00 = ap_pool.tile([128, RT, 2, 512], BF16, tag="at")
        at01 = ap_pool.tile([128, RT, 2, 512], BF16, tag="at")

        def attn_v_norm(po, p, ch, hi):
            # broadcast denom (PSUM row 64) to 64 partitions, then
            # fast-approx reciprocal and one fused multiply-store
            rc = sm_pool.tile([64, 512], F32, tag="rc")
            nc.vector.stream_shuffle(out=rc[0:32, :], in_=po[64:96, :],
                                     mask=[0] * 32)
            nc.vector.stream_shuffle(out=rc[32:64, :], in_=po[64:96, :],
                                     mask=[0] * 32)
            rcr = sm_pool.tile([64, 512], F32, tag="rcr")
            nc.vector.reciprocal_approx_fast(out=rcr[:], in_=rc[:])
            nc.vector.tensor_tensor(
                out=ot[hi * DH:(hi + 1) * DH,
                       p * N + ch * 512:p * N + (ch + 1) * 512],
                in0=po[0:64, :], in1=rcr[:], op=mybir.AluOpType.mult)

        def attn_v_mms(po, at, p, ch, hi, t0, t1):
            h = 2 * p + hi
            for t in range(t0, t1):
                # stationary is a 128-col view [V_h | ones | junk]
                # (FWL path); PSUM rows 65:128 are never read
                nc.tensor.matmul(
                    po[:],
                    v_sb[:, t, h * 65:h * 65 + 128],
                    at[:, t, hi, :],
                    start=(t == 0), stop=(t == RT - 1))

        with ExitStack() as ph1:
            xp = ph1.enter_context(tc.tile_pool(name="xp", bufs=4))
            xnp = ph1.enter_context(tc.tile_pool(name="xnp", bufs=3))
            stat = ph1.enter_context(tc.tile_pool(name="stat", bufs=4))
            wvp = ph1.enter_context(tc.tile_pool(name="wvp", bufs=1))
            pst = ph1.enter_context(
                tc.tile_pool(name="pst", bufs=2, space=bass.MemorySpace.PSUM))


            early_x = []
            for rt in range(4):
                x_t = xp.tile([128, DIM], BF16, tag="x")
                # split across DMA queues: one 256KB tile on a single
                # queue takes ~11us; halves land in ~6. The second half
                # goes through the idle GPSIMD sequencer so the Sync
                # queue's descriptor generation doesn't serialize.
                nc.sync.dma_start(x_t[:, 0:512],
                                  x_d[rt * 128:(rt + 1) * 128, 0:512])
                nc.gpsimd.dma_start(x_t[:, 512:1024],
                                    x_d[rt * 128:(rt + 1) * 128, 512:1024])
                early_x.append(x_t)

            # ~3.4us of dummy matmuls while the first x DMA is in
            # flight: the PE HAM clock gate needs one busy SHORT window
            # before it releases 2.4 GHz, so warm it on garbage instead
            # of on the first real transposes/projections
            pewarm = pools["psm"].tile([128, 512], F32, tag="mm")
            for _ in range(64):
                nc.tensor.matmul(pewarm[:, 0:64], ident[:], ident[:, 0:64],
                                 start=True, stop=True)

            wv_sb = wvp.tile([128, KT, 512], BF16, tag="wv")
            for k in range(KT):
                nc.sync.dma_start(wv_sb[:, k, :],
                                  wqkv_d[k * 128:(k + 1) * 128, 1024:1536])
            wqk0 = stage_weights(0)
            pair0 = alloc_pair()

            def ln_rt(rt):
                # LN chain for one row-tile -> bf16 xn tile
                if rt < 4:
                    x_t = early_x[rt]
                else:
                    x_t = xp.tile([128, DIM], BF16, tag="x")
                    nc.sync.dma_start(x_t[:, 0:512],
                                      x_d[rt * 128:(rt + 1) * 128, 0:512])
                    nc.gpsimd.dma_start(x_t[:, 512:1024],
                                        x_d[rt * 128:(rt + 1) * 128,
                                            512:1024])

                stats = stat.tile([128, 2, 6], F32, tag="bnst")
                xr = x_t[:].rearrange("p (s f) -> p s f", s=2)
                for s in range(2):
                    nc.vector.bn_stats(out=stats[:, s, :], in_=xr[:, s, :])
                mv = stat.tile([128, 2], F32, tag="bnag")
                nc.vector.bn_aggr(out=mv[:], in_=stats[:])
                rstd = stat.tile([128, 1], F32, tag="rstd")
                nc.scalar.activation(out=rstd[:], in_=mv[:, 1:2], func=AF.Sqrt,
                                     bias=eps_t[:], scale=1.0)
                nc.vector.reciprocal(out=rstd[:], in_=rstd[:])

                # NOTE: tried GPSIMD for this (idle engine) -- its SW
                # tensor_scalar runs ~30x slower than DVE (14.8us/tile).
                xn_t = xnp.tile([128, DIM], BF16, tag="xn")
                nc.vector.tensor_scalar(
                    out=xn_t[:], in0=x_t[:], scalar1=mv[:, 0:1],
                    scalar2=rstd[:], op0=mybir.AluOpType.subtract,
                    op1=mybir.AluOpType.mult)
                return xn_t

            # LN runs two row-tiles ahead of the PE (transposes never
            # wait on the serial DVE->ACT->DVE LayerNorm chain, even
            # during the DMA-queue ramp)
            ln_q = [ln_rt(0), ln_rt(1)]
            for rt in range(RT):
                xn_t = ln_q.pop(0)
                if rt + 2 < RT:
                    ln_q.append(ln_rt(rt + 2))

                # 8 transposes share one PSUM bank; one wide copy drains
                # them all (fewer DVE ops -> phase 1 is DVE-limited)
                ps = pst.tile([128, KT, 128], BF16, tag="tr")
                for k in range(KT):
                    nc.tensor.transpose(ps[:, k, :],
                                        xn_t[:, k * 128:(k + 1) * 128],
                                        ident[:])
                if rt < 3:
                    # ScalarE is idle before the pair-0 exps start
                    nc.scalar.copy(
                        out=xnt[:, :, rt * 128:(rt + 1) * 128], in_=ps[:])
                else:
                    nc.vector.tensor_copy(
                        out=xnt[:, :, rt * 128:(rt + 1) * 128], in_=ps[:])

                # pair-0 Q/K projections staggered in as rows become
                # ready, then that milestone's 4 dots/exps for chunk
                # (0,0) -- dots t only needs K-chunk t//4, so the first
                # exps start at rt 3. 4-exp blocks (not 1/rt): each
                # Sqrt<->Exp switch costs a ~1.3us ACT table reload, and
                # the 2-ahead LN pipeline absorbs the queue delay.
                if rt in (3, 7, 11, 15):
                    j = rt // 4
                    proj_k_chunk(wqk0, pair0[1], j)
                    proj_q_chunk(wqk0, pair0[0], j)
                    for t in range(4 * j, 4 * j + 4):
                        dots_exp(pair0[0], pair0[1], at00, 0, t)
                if rt == 15:
                    # also start chunk (0,1): its first two dots/exps
                    # here let rt15's V-proj absorb the psd-rotation
                    # wait that otherwise stalls the phase-2 entry
                    for t in range(2):
                        dots_exp(pair0[0], pair0[1], at01, 1, t)

                # V rows for this row-tile (needs only this rt of xnt):
                # one N=512 chain covers all 8 heads
                psv = pools["psm"].tile([128, 512], F32, tag="mm")
                for k in range(KT):
                    nc.tensor.matmul(
                        psv[:],
                        xnt[:, k, rt * 128:(rt + 1) * 128],
                        wv_sb[:, k, :],
                        start=(k == 0), stop=(k == KT - 1))
                psvr = psv[:].rearrange("p (h d) -> p h d", d=DH)
                vdst = v_sb[:, rt, 0:MYH * 65].rearrange(
                    "p (h d) -> p h d", d=65)
                if rt in (3, 7, 11, 15):
                    # milestone rts run exps on ScalarE; keep the V
                    # drain off it there (DVE), ScalarE otherwise
                    nc.vector.tensor_copy(out=vdst[:, :, 0:DH], in_=psvr[:])
                else:
                    nc.scalar.copy(out=vdst[:, :, 0:DH], in_=psvr[:])
            # ones column (softmax denominator) + finite pad for the
            # 128-wide stationary reads of head 7
            nc.gpsimd.memset(
                v_sb[:, :, 0:MYH * 65].rearrange(
                    "p r (h d) -> p r h d", d=65)[:, :, :, 64:65], 1.0)
            nc.gpsimd.memset(v_sb[:, :, MYH * 65:VW], 0.0)

        # opened only now: ot/wout do not count against the phase-1
        # SBUF peak (pool space is reserved at pool open)
        otp = outer.enter_context(tc.tile_pool(name="otp", bufs=1))
        ot = otp.tile([128, PAIRS * N], BF16, tag="ot")
        wout_sb = otp.tile([128, KO, DIM], BF16, tag="wout")
        sm_pool = outer.enter_context(tc.tile_pool(name="smal", bufs=2))
        for k in range(KO):
            nc.sync.dma_start(wout_sb[:, k, :],
                              wout_d[k * 128:(k + 1) * 128, :])



        # ---- phase 2+3: chunk pipeline ----
        # 16 chunks C[i] = (pair, ch). dots+exp for C[i+1] are emitted
        # interleaved with attn@V for C[i] (whose exps finished a full
        # chunk ago) and with the next pair's Q/K projections, so the
        # in-order PE stream never waits on ScalarE.
        with ExitStack() as att:
            pso = att.enter_context(
                tc.tile_pool(name="pso", bufs=2, space=bass.MemorySpace.PSUM))

            def proj_chunk_ops(wqk, dst, wofs, ch):
                ps = pools["psm"].tile([128, 512], F32, tag="mm")
                ops = []
                for k in range(KT):
                    def mm(k=k, ps=ps):
                        nc.tensor.matmul(
                            ps[:], wqk[:, k, wofs:wofs + 128],
                            xnt[:, k, ch * 512:(ch + 1) * 512],
                            start=(k == 0), stop=(k == KT - 1))
                    ops.append(mm)

                def cp(ps=ps):
                    nc.vector.tensor_copy(
                        out=dst[:, ch * 512:(ch + 1) * 512], in_=ps[:])
                ops.append(cp)
                return ops

            def outproj_ops(m, last=False):
                # op-list for one 128-row block of the out-projection
                orow_t = sm_pool.tile([128, DIM], BF16, tag="orow")
                ops = []
                for ch in range(2):
                    ps = pools["psm"].tile([128, 512], F32, tag="mm")
                    for k in range(KO):
                        def mm(k=k, ps=ps, ch=ch):
                            nc.tensor.matmul(
                                ps[:],
                                ot[:, k * N + m * 128:
                                   k * N + (m + 1) * 128],
                                wout_sb[:, k, ch * 512:(ch + 1) * 512],
                                start=(k == 0), stop=(k == KO - 1))
                        ops.append(mm)

                    def cpdma(ps=ps, ch=ch):
                        # the exp stream is over by the time the last
                        # out-proj blocks drain; use the freed ScalarE
                        eng = nc.scalar.copy if last else (
                            lambda out, in_: nc.vector.tensor_copy(
                                out=out, in_=in_))
                        eng(out=orow_t[:, ch * 512:(ch + 1) * 512],
                            in_=ps[:])
                        if last:
                            # tail blocks: split across two queues --
                            # the final DMA's latency IS the kernel tail
                            nc.sync.dma_start(
                                out_d[m * 128:(m + 1) * 128,
                                      ch * 512:ch * 512 + 256],
                                orow_t[:, ch * 512:ch * 512 + 256])
                            nc.gpsimd.dma_start(
                                out_d[m * 128:(m + 1) * 128,
                                      ch * 512 + 256:(ch + 1) * 512],
                                orow_t[:, ch * 512 + 256:(ch + 1) * 512])
                        else:
                            nc.sync.dma_start(
                                out_d[m * 128:(m + 1) * 128,
                                      ch * 512:(ch + 1) * 512],
                                orow_t[:, ch * 512:(ch + 1) * 512])
                    ops.append(cpdma)
                return ops

            chunks = [(p, ch) for p in range(PAIRS) for ch in range(CHS)]
            wqk1 = stage_weights(1)
            pair_tiles = {0: pair0,               # p -> (qt, kt[, wqk])
                          1: alloc_pair() + (wqk1,)}
            at_tiles = {(0, 0): at00, (0, 1): at01}
            # out-proj row-block m needs ot columns m*128 for ALL pairs:
            # blocks 4ch..4ch+3 unlock after pair-3 chunk ch's norms.
            # Emitted interleaved into the following chunk's t-loop.
            outproj_pending = []

            for i in range(len(chunks)):
                ac = chunks[i]                    # attn@V chunk (exps done)
                dc = chunks[i + 1] if i + 1 < len(chunks) else None
                pending = []
                if dc is not None:
                    dp = dc[0]
                    if dc[1] == 0 and dp + 1 < PAIRS:
                        # entering pair dp: stage weights+tiles for dp+1
                        wqk_n = stage_weights(dp + 1)
                        pair_tiles[dp + 1] = alloc_pair() + (wqk_n,)
                    if dp + 1 < PAIRS:
                        nq_t, nk_t, wqk_n = pair_tiles[dp + 1]
                        proj_chs = [0, 1] if i == 0 else [dc[1]]
                        for pc in proj_chs:
                            pending += proj_chunk_ops(wqk_n, nk_t, 128, pc)
                            pending += proj_chunk_ops(wqk_n, nq_t, 0, pc)
                    dq_p, dk_p = pair_tiles[dp][0], pair_tiles[dp][1]
                    at_d = at_tiles.get(dc)
                    if at_d is None:
                        at_d = ap_pool.tile([128, RT, 2, 512], BF16,
                                            tag="at")
                        at_tiles[dc] = at_d
                at_a = at_tiles.pop(ac)
                po0 = pso.tile([128, 512], F32, tag="po")
                po1 = None
                pending += outproj_pending
                outproj_pending = []

                for t in range(RT):
                    # attn@V FIRST, dots after: the attn@V stationary
                    # (128x128, FWL) background-loads behind the
                    # preceding full-row matmul, but not behind the
                    # K=64 dots pair -- ordering av before dots avoids
                    # one ~108ns LDWEIGHTS stall per t-step.
                    # attn@V for ac: hi=0 front-loaded (t<8), hi=1 after
                    if t < 8:
                        attn_v_mms(po0, at_a, ac[0], ac[1], 0,
                                   2 * t, 2 * t + 2)
                        if t == 7:
                            attn_v_norm(po0, ac[0], ac[1], 0)
                    else:
                        if t == 8:
                            po1 = pso.tile([128, 512], F32, tag="po")
                        attn_v_mms(po1, at_a, ac[0], ac[1], 1,
                                   2 * (t - 8), 2 * (t - 8) + 2)
                        if t == RT - 1:
                            attn_v_norm(po1, ac[0], ac[1], 1)
                    if dc is not None and not (dc == (0, 1) and t < 2):
                        dots_exp(dq_p, dk_p, at_d, dc[1], t)
                    if t >= 2 and pending:
                        pending.pop(0)()
                        if pending:
                            pending.pop(0)()
                        if dc is None and pending:
                            pending.pop(0)()
                while pending:
                    pending.pop(0)()

                if ac[0] == PAIRS - 1:
                    # pair-3 chunk done: out-proj rows ac[1]*512 +
                    # 0:512 are fully normalized; queue their 4 blocks
                    for m in range(4 * ac[1], 4 * ac[1] + 4):
                        outproj_pending += outproj_ops(
                            m, last=(ac[1] == CHS - 1))

            # ---- tail: remaining out-proj blocks (last chunk's rows)
            while outproj_pending:
                outproj_pending.pop(0)()


_NC_CACHE = None


def _make_in_maps(x, wqkv_bf, wout_bf):
    in_maps = []
    for core in range(N_CORES):
        b, hh = core // 2, core % 2
        # per-core w_qkv slice: q/k/v columns of this core's 8 heads
        w_sl = np.concatenate(
            [wqkv_bf[:, hh * 512:(hh + 1) * 512],
             wqkv_bf[:, INNER + hh * 512:INNER + (hh + 1) * 512],
             wqkv_bf[:, 2 * INNER + hh * 512:2 * INNER + (hh + 1) * 512]],
            axis=1)
        in_maps.append({
            "x": np.ascontiguousarray(x[b]).astype(ml_dtypes.bfloat16),
            "wqkv": np.ascontiguousarray(w_sl),
            "wout": np.ascontiguousarray(wout_bf[hh * 512:(hh + 1) * 512, :]),
        })
    return in_maps


def kernel(x, ln_gamma, ln_beta, w_qkv, w_out, b_out):
    global _NC_CACHE
    x = np.asarray(x, dtype=np.float32)
    ln_gamma = np.asarray(ln_gamma, dtype=np.float32)
    ln_beta = np.asarray(ln_beta, dtype=np.float32)
    w_qkv = np.asarray(w_qkv, dtype=np.float32)
    w_out = np.asarray(w_out, dtype=np.float32)
    b_out = np.asarray(b_out, dtype=np.float32)

    # fold gamma + softmax scale into w_qkv (host, exact f32)
    wqkv_eff = w_qkv * ln_gamma[:, None]
    wqkv_eff = wqkv_eff.copy()
    wqkv_eff[:, :INNER] *= SCALE
    qkv_bias = ln_beta @ w_qkv
    assert not np.any(qkv_bias), "nonzero ln_beta not supported on device"
    wqkv_bf = wqkv_eff.astype(ml_dtypes.bfloat16)
    wout_bf = w_out.astype(ml_dtypes.bfloat16)

    if _NC_CACHE is None:
        _NC_CACHE = _build_graph()
    nc = _NC_CACHE

    # clear any wedged NRT state left by a previous process on the cores
    try:
        import ctypes
        import jax
        jax.devices()
        _lib = ctypes.CDLL("/opt/axon/libaxon_pjrt.so")
        if hasattr(_lib, "axon_reset"):
            _lib.axon_reset.restype = ctypes.c_int64
            _lib.axon_reset()
    except Exception:
        pass

    in_maps = _make_in_maps(x, wqkv_bf, wout_bf)
    res = run_bass_kernel_spmd(nc, in_maps, core_ids=list(range(N_CORES)))

    out = np.empty((B, N, DIM), dtype=np.float32)
    for b in range(B):
        out[b] = np.asarray(res.results[2 * b]["out"], dtype=np.float32)
        out[b] += np.asarray(res.results[2 * b + 1]["out"],
                             dtype=np.float32)
    out += b_out
    return out


# revision 59
# speedup vs baseline: 1.1922x; 1.1922x over previous
"""Multi-head attention block (LN -> QKV -> attention -> out-proj) on 8 TRN2 cores.

Sharding: (batch, head-half). Core i handles batch i//2 and heads
8*(i%2) .. 8*(i%2)+8 for ALL 2048 query rows. Each core computes LN for
its whole batch, Q/K/V for its 8 heads, attention, and a PARTIAL
out-projection (its heads' slice of w_out rows). The host sums the two
partial outputs per batch (+ b_out). This removes the K/V-projection
duplication of a (batch, query-half) split: per-core projection work
drops from 768 to 512 N=512 matmuls.

All 8 cores run ONE graph; per-core inputs differ (x per batch, weight
slices per head-half).

Compute dtype bf16 (PSUM accumulation f32). Host folds ln_gamma and the
softmax scale into w_qkv (exact f32); ln_beta @ w_qkv is asserted zero.

Attention data path (per head pair, per 512-query chunk):
  dots^T = K @ Q^T as two K=64 row-tiled matmuls (tile rows 0/64) into
  ONE [128, 1024] PSUM tile (adjacent banks) -> a single 1024-wide exp
  on ScalarE writes attn^T bf16.
  attn@V uses V_aug as the STATIONARY operand with a 128-COLUMN access
  pattern (cols = [V_h | ones | next head's data / pad]) so the fast
  weight load path (FWL needs a 128x128 stationary) keeps LDWEIGHTS off
  the critical path; PSUM rows 65:128 accumulate junk that is never
  read. Row 64 accumulates the softmax denominator.
  The denominator row is broadcast to 64 partitions with two DVE
  stream_shuffles, reciprocal'd with the fast approx op (~5x cheaper
  than exact; denominators are O(1e2..1e3), far from its edge cases),
  and multiplied into the out rows by one tensor_tensor -> ot tile.
  attn@V for chunk c is emitted interleaved into the dots/exp loop of
  chunk c+1, so the PE never waits on the exp stream.

Partial outputs are written bf16 (halves the 8MB/core output DMA; the
host sums the two per-batch partials in f32 -- the added rounding is
~0.4% RMS against a 2e-2 budget). Steady state runs the PE at ~99%
and ScalarE (exp) at ~97% concurrently; both engines are at their
structural floor (PE: 512 proj + 512 attn@V + 256 dots-pair matmul
slots, all N=512 streams; ScalarE: 256 1024-wide exps).
"""

import sys

sys.path.insert(0, "/opt/trn_rl_repo")

import ml_dtypes
import numpy as np

import concourse.bass as bass
import concourse.tile as tile
from concourse import bacc, mybir
from concourse.bass_utils import run_bass_kernel_spmd
from concourse.masks import make_identity

F32 = mybir.dt.float32
BF16 = mybir.dt.bfloat16
AF = mybir.ActivationFunctionType

B, N, DIM = 4, 2048, 1024
HEADS, DH = 16, 64
INNER = HEADS * DH  # 1024
SCALE = DH ** -0.5
N_CORES = 8
RT = N // 128         # 16 row tiles
KT = DIM // 128       # 8 contraction tiles (qkv proj)
MYH = HEADS // 2      # 8 heads per core
PAIRS = MYH // 2      # 4 head pairs per core
CHS = N // 512        # 4 query chunks of 512
KO = MYH * DH // 128  # 4 contraction tiles (out proj)
VW = MYH * 65 + 63    # v_sb free width per row-tile (65-per-head + pad)
EPS = 1e-5


def _build_graph():
    nc = bacc.Bacc("TRN2", target_bir_lowering=False, debug=False,
                   num_devices=N_CORES)
    x_d = nc.dram_tensor("x", [N, DIM], BF16, kind="ExternalInput").ap()
    # per-core slice: [q 512 | k 512 | v 512] columns of w_qkv
    wqkv_d = nc.dram_tensor("wqkv", [DIM, 3 * 512], BF16,
                            kind="ExternalInput").ap()
    # per-core slice: rows hh*512:(hh+1)*512 of w_out
    wout_d = nc.dram_tensor("wout", [512, DIM], BF16,
                            kind="ExternalInput").ap()
    # bf16 partial outputs: the host sums the two per-batch partials in
    # f32; the extra ~0.4% rounding is far inside the error budget and
    # halves the output DMA (8MB -> 4MB per core)
    out_d = nc.dram_tensor("out", [N, DIM], BF16, kind="ExternalOutput").ap()

    with tile.TileContext(nc) as tc:
        _kernel_body(tc, x_d, wqkv_d, wout_d, out_d)
    nc.compile()
    return nc


def _kernel_body(tc, x_d, wqkv_d, wout_d, out_d):
    nc = tc.nc
    from contextlib import ExitStack

    with ExitStack() as outer:
        const_pool = outer.enter_context(tc.tile_pool(name="const", bufs=1))
        persist = outer.enter_context(tc.tile_pool(name="persist", bufs=1))
        # NOTE: denser PE packing (3 dots buffers + 2-t grouping) was
        # tried and REGRESSED: the sustained matmul density trips the
        # chip's activity throttle / HAM oscillations. The ~107ns
        # contraction-row-switch stalls per t are effectively free
        # power headroom. Keep 2+2+2+2 PSUM pools and per-t interleave.
        pools = {}
        pools["psm"] = outer.enter_context(
            tc.tile_pool(name="psm", bufs=2, space=bass.MemorySpace.PSUM))
        pools["psd"] = outer.enter_context(
            tc.tile_pool(name="psd", bufs=2, space=bass.MemorySpace.PSUM))

        ident = const_pool.tile([128, 128], BF16, tag="ident")
        make_identity(nc, ident[:])
        eps_t = const_pool.tile([128, 1], F32, tag="eps")
        nc.gpsimd.memset(eps_t[:], EPS)
        # preload the Sqrt and Exp activation tables with dummy ops while
        # the first x DMA is still in flight -- the first real rstd/exp
        # otherwise pays a ~1.3us ACT_TABLE_LOAD on the critical path
        warm_t = const_pool.tile([1, 1], F32, tag="actwarm")
        nc.scalar.activation(out=warm_t[:], in_=eps_t[0:1, :], func=AF.Sqrt)
        nc.scalar.activation(out=warm_t[:], in_=eps_t[0:1, :], func=AF.Exp)


        # persistent through the whole kernel
        xnt = persist.tile([128, KT, N], BF16, tag="xnt")      # [dim, kt, row]
        v_sb = persist.tile([128, RT, VW], BF16, tag="v")      # [kv,rt,h*65+d]

        # per-pair projection staging (lives through the pipeline)
        wqkp = outer.enter_context(tc.tile_pool(name="wqkp", bufs=2))
        qtp_pool = outer.enter_context(tc.tile_pool(name="qtp", bufs=2))
        ktp_pool = outer.enter_context(tc.tile_pool(name="ktp", bufs=2))
        ap_pool = outer.enter_context(tc.tile_pool(name="attn", bufs=2))

        def stage_weights(p):
            wqk = wqkp.tile([128, KT, 256], BF16, tag="wqk")
            for k in range(KT):
                nc.sync.dma_start(
                    wqk[:, k, 0:128],
                    wqkv_d[k * 128:(k + 1) * 128, p * 128:(p + 1) * 128])
                nc.sync.dma_start(
                    wqk[:, k, 128:256],
                    wqkv_d[k * 128:(k + 1) * 128,
                           512 + p * 128:512 + (p + 1) * 128])
            return wqk

        def proj_q_chunk(wqk, qt_p, ch):
            ps = pools["psm"].tile([128, 512], F32, tag="mm")
            for k in range(KT):
                nc.tensor.matmul(
                    ps[:], wqk[:, k, 0:128],
                    xnt[:, k, ch * 512:(ch + 1) * 512],
                    start=(k == 0), stop=(k == KT - 1))
            nc.vector.tensor_copy(out=qt_p[:, ch * 512:(ch + 1) * 512],
                                  in_=ps[:])

        def proj_k_chunk(wqk, kt_p, ch):
            ps = pools["psm"].tile([128, 512], F32, tag="mm")
            for k in range(KT):
                nc.tensor.matmul(
                    ps[:], wqk[:, k, 128:256],
                    xnt[:, k, ch * 512:(ch + 1) * 512],
                    start=(k == 0), stop=(k == KT - 1))
            nc.vector.tensor_copy(out=kt_p[:, ch * 512:(ch + 1) * 512],
                                  in_=ps[:])

        def alloc_pair():
            qt_p = qtp_pool.tile([128, N], BF16, tag="qt")
            kt_p = ktp_pool.tile([128, N], BF16, tag="kt")
            return qt_p, kt_p

        def dots_exp(qt_p, kt_p, at, ch, t):
            # both heads of the pair into one [128,1024] psum (2 banks),
            # K=64 row tiles 0/64 run concurrently in the PE array.
            # NOTE: a bf16-PSUM variant via is_transpose (1-bank tiles,
            # 2-t grouping + 2048-wide exps) compiles but crashes the
            # device at runtime -- TRN2 matmul output must be fp32.
            ps = pools["psd"].tile([128, 1024], F32, tag="dots")
            for hi in range(2):
                nc.tensor.matmul(
                    ps[:, hi * 512:(hi + 1) * 512],
                    kt_p[hi * DH:(hi + 1) * DH, t * 128:(t + 1) * 128],
                    qt_p[hi * DH:(hi + 1) * DH, ch * 512:(ch + 1) * 512],
                    start=True, stop=True)
            nc.scalar.activation(out=at[:, t, :, :], in_=ps[:], func=AF.Exp)

        # ---- phase 1: LayerNorm + transpose into xnt; V projection ----
        # x row-tiles 0..3 are DMA'd before any weight staging so LN and
        # the PE transposes start within ~2us of kernel entry.
        at00 = ap_pool.tile([128, RT, 2, 512], BF16, tag="at")
        at01 = ap_pool.tile([128, RT, 2, 512], BF16, tag="at")

        def attn_v_norm(po, p, ch, hi):
            # broadcast denom (PSUM row 64) to 64 partitions, then
            # fast-approx reciprocal and one fused multiply-store
            rc = sm_pool.tile([64, 512], F32, tag="rc")
            nc.vector.stream_shuffle(out=rc[0:32, :], in_=po[64:96, :],
                                     mask=[0] * 32)
            nc.vector.stream_shuffle(out=rc[32:64, :], in_=po[64:96, :],
                                     mask=[0] * 32)
            rcr = sm_pool.tile([64, 512], F32, tag="rcr")
            nc.vector.reciprocal_approx_fast(out=rcr[:], in_=rc[:])
            nc.vector.tensor_tensor(
                out=ot[hi * DH:(hi + 1) * DH,
                       p * N + ch * 512:p * N + (ch + 1) * 512],
                in0=po[0:64, :], in1=rcr[:], op=mybir.AluOpType.mult)

        def attn_v_mms(po, at, p, ch, hi, t0, t1):
            h = 2 * p + hi
            for t in range(t0, t1):
                # stationary is a 128-col view [V_h | ones | junk]
                # (FWL path); PSUM rows 65:128 are never read
                nc.tensor.matmul(
                    po[:],
                    v_sb[:, t, h * 65:h * 65 + 128],
                    at[:, t, hi, :],
                    start=(t == 0), stop=(t == RT - 1))

        with ExitStack() as ph1:
            xp = ph1.enter_context(tc.tile_pool(name="xp", bufs=4))
            xnp = ph1.enter_context(tc.tile_pool(name="xnp", bufs=3))
            stat = ph1.enter_context(tc.tile_pool(name="stat", bufs=4))
            wvp = ph1.enter_context(tc.tile_pool(name="wvp", bufs=1))
            pst = ph1.enter_context(
                tc.tile_pool(name="pst", bufs=2, space=bass.MemorySpace.PSUM))


            early_x = []
            for rt in range(4):
                x_t = xp.tile([128, DIM], BF16, tag="x")
                # split across DMA queues: one 256KB tile on a single
                # queue takes ~11us; halves land in ~6. The second half
                # goes through the idle GPSIMD sequencer so the Sync
                # queue's descriptor generation doesn't serialize.
                nc.sync.dma_start(x_t[:, 0:512],
                                  x_d[rt * 128:(rt + 1) * 128, 0:512])
                nc.gpsimd.dma_start(x_t[:, 512:1024],
                                    x_d[rt * 128:(rt + 1) * 128, 512:1024])
                early_x.append(x_t)

            # ~3.4us of dummy matmuls while the first x DMA is in
            # flight: the PE HAM clock gate needs one busy SHORT window
            # before it releases 2.4 GHz, so warm it on garbage instead
            # of on the first real transposes/projections
            pewarm = pools["psm"].tile([128, 512], F32, tag="mm")
            for _ in range(64):
                nc.tensor.matmul(pewarm[:, 0:64], ident[:], ident[:, 0:64],
                                 start=True, stop=True)

            wv_sb = wvp.tile([128, KT, 512], BF16, tag="wv")
            for k in range(KT):
                nc.sync.dma_start(wv_sb[:, k, :],
                                  wqkv_d[k * 128:(k + 1) * 128, 1024:1536])
            wqk0 = stage_weights(0)
            pair0 = alloc_pair()

            def ln_rt(rt):
                # LN chain for one row-tile -> bf16 xn tile
                if rt < 4:
                    x_t = early_x[rt]
                else:
                    x_t = xp.tile([128, DIM], BF16, tag="x")
                    nc.sync.dma_start(x_t[:, 0:512],
                                      x_d[rt * 128:(rt + 1) * 128, 0:512])
                    nc.gpsimd.dma_start(x_t[:, 512:1024],
                                        x_d[rt * 128:(rt + 1) * 128,
                                            512:1024])

                stats = stat.tile([128, 2, 6], F32, tag="bnst")
                xr = x_t[:].rearrange("p (s f) -> p s f", s=2)
                for s in range(2):
                    nc.vector.bn_stats(out=stats[:, s, :], in_=xr[:, s, :])
                mv = stat.tile([128, 2], F32, tag="bnag")
                nc.vector.bn_aggr(out=mv[:], in_=stats[:])
                rstd = stat.tile([128, 1], F32, tag="rstd")
                nc.scalar.activation(out=rstd[:], in_=mv[:, 1:2], func=AF.Sqrt,
                                     bias=eps_t[:], scale=1.0)
                nc.vector.reciprocal(out=rstd[:], in_=rstd[:])

                # NOTE: tried GPSIMD for this (idle engine) -- its SW
                # tensor_scalar runs ~30x slower than DVE (14.8us/tile).
                xn_t = xnp.tile([128, DIM], BF16, tag="xn")
                nc.vector.tensor_scalar(
                    out=xn_t[:], in0=x_t[:], scalar1=mv[:, 0:1],
                    scalar2=rstd[:], op0=mybir.AluOpType.subtract,
                    op1=mybir.AluOpType.mult)
                return xn_t

            # LN runs two row-tiles ahead of the PE (transposes never
            # wait on the serial DVE->ACT->DVE LayerNorm chain, even
            # during the DMA-queue ramp)
            ln_q = [ln_rt(0), ln_rt(1)]
            for rt in range(RT):
                xn_t = ln_q.pop(0)
                if rt + 2 < RT:
                    ln_q.append(ln_rt(rt + 2))

                # 8 transposes share one PSUM bank; one wide copy drains
                # them all (fewer DVE ops -> phase 1 is DVE-limited)
                ps = pst.tile([128, KT, 128], BF16, tag="tr")
                for k in range(KT):
                    nc.tensor.transpose(ps[:, k, :],
                                        xn_t[:, k * 128:(k + 1) * 128],
                                        ident[:])
                if rt < 3:
                    # ScalarE is idle before the pair-0 exps start
                    nc.scalar.copy(
                        out=xnt[:, :, rt * 128:(rt + 1) * 128], in_=ps[:])
                else:
                    nc.vector.tensor_copy(
                        out=xnt[:, :, rt * 128:(rt + 1) * 128], in_=ps[:])

                # pair-0 Q/K projections staggered in as rows become
                # ready, then that milestone's 4 dots/exps for chunk
                # (0,0) -- dots t only needs K-chunk t//4, so the first
                # exps start at rt 3. 4-exp blocks (not 1/rt): each
                # Sqrt<->Exp switch costs a ~1.3us ACT table reload, and
                # the 2-ahead LN pipeline absorbs the queue delay.
                if rt in (3, 7, 11, 15):
                    j = rt // 4
                    proj_k_chunk(wqk0, pair0[1], j)
                    proj_q_chunk(wqk0, pair0[0], j)
                    for t in range(4 * j, 4 * j + 4):
                        dots_exp(pair0[0], pair0[1], at00, 0, t)
                if rt == 15:
                    # also start chunk (0,1): its first two dots/exps
                    # here let rt15's V-proj absorb the psd-rotation
                    # wait that otherwise stalls the phase-2 entry
                    for t in range(2):
                        dots_exp(pair0[0], pair0[1], at01, 1, t)

                # V rows for this row-tile (needs only this rt of xnt):
                # one N=512 chain covers all 8 heads
                psv = pools["psm"].tile([128, 512], F32, tag="mm")
                for k in range(KT):
                    nc.tensor.matmul(
                        psv[:],
                        xnt[:, k, rt * 128:(rt + 1) * 128],
                        wv_sb[:, k, :],
                        start=(k == 0), stop=(k == KT - 1))
                psvr = psv[:].rearrange("p (h d) -> p h d", d=DH)
                vdst = v_sb[:, rt, 0:MYH * 65].rearrange(
                    "p (h d) -> p h d", d=65)
                if rt in (3, 7, 11, 15):
                    # milestone rts run exps on ScalarE; keep the V
                    # drain off it there (DVE), ScalarE otherwise
                    nc.vector.tensor_copy(out=vdst[:, :, 0:DH], in_=psvr[:])
                else:
                    nc.scalar.copy(out=vdst[:, :, 0:DH], in_=psvr[:])
            # ones column (softmax denominator) + finite pad for the
            # 128-wide stationary reads of head 7
            nc.gpsimd.memset(
                v_sb[:, :, 0:MYH * 65].rearrange(
                    "p r (h d) -> p r h d", d=65)[:, :, :, 64:65], 1.0)
            nc.gpsimd.memset(v_sb[:, :, MYH * 65:VW], 0.0)

        # opened only now: ot/wout do not count against the phase-1
        # SBUF peak (pool space is reserved at pool open)
        otp = outer.enter_context(tc.tile_pool(name="otp", bufs=1))
        ot = otp.tile([128, PAIRS * N], BF16, tag="ot")
        wout_sb = otp.tile([128, KO, DIM], BF16, tag="wout")
        sm_pool = outer.enter_context(tc.tile_pool(name="smal", bufs=2))
        for k in range(KO):
            nc.sync.dma_start(wout_sb[:, k, :],
                              wout_d[k * 128:(k + 1) * 128, :])



        # ---- phase 2+3: chunk pipeline ----
        # 16 chunks C[i] = (pair, ch). dots+exp for C[i+1] are emitted
        # interleaved with attn@V for C[i] (whose exps finished a full
        # chunk ago) and with the next pair's Q/K projections, so the
        # in-order PE stream never waits on ScalarE.
        with ExitStack() as att:
            pso = att.enter_context(
                tc.tile_pool(name="pso", bufs=2, space=bass.MemorySpace.PSUM))

            def proj_chunk_ops(wqk, dst, wofs, ch):
                ps = pools["psm"].tile([128, 512], F32, tag="mm")
                ops = []
                for k in range(KT):
                    def mm(k=k, ps=ps):
                        nc.tensor.matmul(
                            ps[:], wqk[:, k, wofs:wofs + 128],
                            xnt[:, k, ch * 512:(ch + 1) * 512],
                            start=(k == 0), stop=(k == KT - 1))
                    ops.append(mm)

                def cp(ps=ps):
                    nc.vector.tensor_copy(
                        out=dst[:, ch * 512:(ch + 1) * 512], in_=ps[:])
                ops.append(cp)
                return ops

            def outproj_ops(m, last=False):
                # op-list for one 128-row block of the out-projection
                orow_t = sm_pool.tile([128, DIM], BF16, tag="orow")
                ops = []
                for ch in range(2):
                    ps = pools["psm"].tile([128, 512], F32, tag="mm")
                    for k in range(KO):
                        def mm(k=k, ps=ps, ch=ch):
                            nc.tensor.matmul(
                                ps[:],
                                ot[:, k * N + m * 128:
                                   k * N + (m + 1) * 128],
                                wout_sb[:, k, ch * 512:(ch + 1) * 512],
                                start=(k == 0), stop=(k == KO - 1))
                        ops.append(mm)

                    def cpdma(ps=ps, ch=ch):
                        # the exp stream is over by the time the last
                        # out-proj blocks drain; use the freed ScalarE
                        eng = nc.scalar.copy if last else (
                            lambda out, in_: nc.vector.tensor_copy(
                                out=out, in_=in_))
                        eng(out=orow_t[:, ch * 512:(ch + 1) * 512],
                            in_=ps[:])
                        if last:
                            # tail blocks: split across two queues --
                            # the final DMA's latency IS the kernel tail
                            nc.sync.dma_start(
                                out_d[m * 128:(m + 1) * 128,
                                      ch * 512:ch * 512 + 256],
                                orow_t[:, ch * 512:ch * 512 + 256])
                            nc.gpsimd.dma_start(
                                out_d[m * 128:(m + 1) * 128,
                                      ch * 512 + 256:(ch + 1) * 512],
                                orow_t[:, ch * 512 + 256:(ch + 1) * 512])
                        else:
                            nc.sync.dma_start(
                                out_d[m * 128:(m + 1) * 128,
                                      ch * 512:(ch + 1) * 512],
                                orow_t[:, ch * 512:(ch + 1) * 512])
                    ops.append(cpdma)
                return ops

            chunks = [(p, ch) for p in range(PAIRS) for ch in range(CHS)]
            wqk1 = stage_weights(1)
            pair_tiles = {0: pair0,               # p -> (qt, kt[, wqk])
                          1: alloc_pair() + (wqk1,)}
            at_tiles = {(0, 0): at00, (0, 1): at01}
            # out-proj row-block m needs ot columns m*128 for ALL pairs:
            # blocks 4ch..4ch+3 unlock after pair-3 chunk ch's norms.
            # Emitted interleaved into the following chunk's t-loop.
            outproj_pending = []

            for i in range(len(chunks)):
                ac = chunks[i]                    # attn@V chunk (exps done)
                dc = chunks[i + 1] if i + 1 < len(chunks) else None
                pending = []
                if dc is not None:
                    dp = dc[0]
                    if dc[1] == 0 and dp + 1 < PAIRS:
                        # entering pair dp: stage weights+tiles for dp+1
                        wqk_n = stage_weights(dp + 1)
                        pair_tiles[dp + 1] = alloc_pair() + (wqk_n,)
                    if dp + 1 < PAIRS:
                        nq_t, nk_t, wqk_n = pair_tiles[dp + 1]
                        proj_chs = [0, 1] if i == 0 else [dc[1]]
                        for pc in proj_chs:
                            pending += proj_chunk_ops(wqk_n, nk_t, 128, pc)
                            pending += proj_chunk_ops(wqk_n, nq_t, 0, pc)
                    dq_p, dk_p = pair_tiles[dp][0], pair_tiles[dp][1]
                    at_d = at_tiles.get(dc)
                    if at_d is None:
                        at_d = ap_pool.tile([128, RT, 2, 512], BF16,
                                            tag="at")
                        at_tiles[dc] = at_d
                at_a = at_tiles.pop(ac)
                po0 = pso.tile([128, 512], F32, tag="po")
                po1 = None
                pending += outproj_pending
                outproj_pending = []

                for t in range(RT):
                    # attn@V FIRST, dots after: the attn@V stationary
                    # (128x128, FWL) background-loads behind the
                    # preceding full-row matmul, but not behind the
                    # K=64 dots pair -- ordering av before dots avoids
                    # one ~108ns LDWEIGHTS stall per t-step.
                    # attn@V for ac: hi=0 front-loaded (t<8), hi=1 after
                    if t < 8:
                        attn_v_mms(po0, at_a, ac[0], ac[1], 0,
                                   2 * t, 2 * t + 2)
                        if t == 7:
                            attn_v_norm(po0, ac[0], ac[1], 0)
                    else:
                        if t == 8:
                            po1 = pso.tile([128, 512], F32, tag="po")
                        attn_v_mms(po1, at_a, ac[0], ac[1], 1,
                                   2 * (t - 8), 2 * (t - 8) + 2)
                        if t == RT - 1:
                            attn_v_norm(po1, ac[0], ac[1], 1)
                    if dc is not None and not (dc == (0, 1) and t < 2):
                        dots_exp(dq_p, dk_p, at_d, dc[1], t)
                    if t >= 2 and pending:
                        pending.pop(0)()
                        if pending:
                            pending.pop(0)()
                        if dc is None and pending:
                            pending.pop(0)()
                while pending:
                    pending.pop(0)()

                if ac[0] == PAIRS - 1:
                    # pair-3 chunk done: out-proj rows ac[1]*512 +
                    # 0:512 are fully normalized; queue their 4 blocks
                    for m in range(4 * ac[1], 4 * ac[1] + 4):
                        outproj_pending += outproj_ops(
                            m, last=(ac[1] == CHS - 1))

            # ---- tail: remaining out-proj blocks (last chunk's rows)
            while outproj_pending:
                outproj_pending.pop(0)()


_NC_CACHE = None


def _make_in_maps(x, wqkv_bf, wout_bf):
    in_maps = []
    for core in range(N_CORES):
        b, hh = core // 2, core % 2
        # per-core w_qkv slice: q/k/v columns of this core's 8 heads
        w_sl = np.concatenate(
            [wqkv_bf[:, hh * 512:(hh + 1) * 512],
             wqkv_bf[:, INNER + hh * 512:INNER + (hh + 1) * 512],
             wqkv_bf[:, 2 * INNER + hh * 512:2 * INNER + (hh + 1) * 512]],
            axis=1)
        in_maps.append({
            "x": np.ascontiguousarray(x[b]).astype(ml_dtypes.bfloat16),
            "wqkv": np.ascontiguousarray(w_sl),
            "wout": np.ascontiguousarray(wout_bf[hh * 512:(hh + 1) * 512, :]),
        })
    return in_maps


def kernel(x, ln_gamma, ln_beta, w_qkv, w_out, b_out):
    global _NC_CACHE
    x = np.asarray(x, dtype=np.float32)
    ln_gamma = np.asarray(ln_gamma, dtype=np.float32)
    ln_beta = np.asarray(ln_beta, dtype=np.float32)
    w_qkv = np.asarray(w_qkv, dtype=np.float32)
    w_out = np.asarray(w_out, dtype=np.float32)
    b_out = np.asarray(b_out, dtype=np.float32)

    # fold gamma + softmax scale into w_qkv (host, exact f32)
    wqkv_eff = w_qkv * ln_gamma[:, None]
    wqkv_eff = wqkv_eff.copy()
    wqkv_eff[:, :INNER] *= SCALE
    qkv_bias = ln_beta @ w_qkv
    assert not np.any(qkv_bias), "nonzero ln_beta not supported on device"
    wqkv_bf = wqkv_eff.astype(ml_dtypes.bfloat16)
    wout_bf = w_out.astype(ml_dtypes.bfloat16)

    if _NC_CACHE is None:
        _NC_CACHE = _build_graph()
    nc = _NC_CACHE

    # clear any wedged NRT state left by a previous process on the cores
    try:
        import ctypes
        import jax
        jax.devices()
        _lib = ctypes.CDLL("/opt/axon/libaxon_pjrt.so")
        if hasattr(_lib, "axon_reset"):
            _lib.axon_reset.restype = ctypes.c_int64
            _lib.axon_reset()
    except Exception:
        pass

    in_maps = _make_in_maps(x, wqkv_bf, wout_bf)
    res = run_bass_kernel_spmd(nc, in_maps, core_ids=list(range(N_CORES)))

    out = np.empty((B, N, DIM), dtype=np.float32)
    for b in range(B):
        out[b] = np.asarray(res.results[2 * b]["out"], dtype=np.float32)
        out[b] += np.asarray(res.results[2 * b + 1]["out"],
                             dtype=np.float32)
    out += b_out
    return out


# revision 61
# speedup vs baseline: 1.1936x; 1.0012x over previous
"""Multi-head attention block (LN -> QKV -> attention -> out-proj) on 8 TRN2 cores.

Sharding: (batch, head-half). Core i handles batch i//2 and heads
8*(i%2) .. 8*(i%2)+8 for ALL 2048 query rows. Each core computes LN for
its whole batch, Q/K/V for its 8 heads, attention, and a PARTIAL
out-projection (its heads' slice of w_out rows). The host sums the two
partial outputs per batch (+ b_out). This removes the K/V-projection
duplication of a (batch, query-half) split: per-core projection work
drops from 768 to 512 N=512 matmuls.

All 8 cores run ONE graph; per-core inputs differ (x per batch, weight
slices per head-half).

Compute dtype bf16 (PSUM accumulation f32). Host folds ln_gamma and the
softmax scale into w_qkv (exact f32); ln_beta @ w_qkv is asserted zero.

Attention data path (per head pair, per 512-query chunk):
  dots^T = K @ Q^T as two K=64 row-tiled matmuls (tile rows 0/64) into
  ONE [128, 1024] PSUM tile (adjacent banks) -> a single 1024-wide exp
  on ScalarE writes attn^T bf16.
  attn@V uses V_aug as the STATIONARY operand with a 128-COLUMN access
  pattern (cols = [V_h | ones | next head's data / pad]) so the fast
  weight load path (FWL needs a 128x128 stationary) keeps LDWEIGHTS off
  the critical path; PSUM rows 65:128 accumulate junk that is never
  read. Row 64 accumulates the softmax denominator.
  The denominator row is broadcast to 64 partitions with two DVE
  stream_shuffles, reciprocal'd with the fast approx op (~5x cheaper
  than exact; denominators are O(1e2..1e3), far from its edge cases),
  and multiplied into the out rows by one tensor_tensor -> ot tile.
  attn@V for chunk c is emitted interleaved into the dots/exp loop of
  chunk c+1, so the PE never waits on the exp stream.

Partial outputs are written bf16 (halves the 8MB/core output DMA; the
host sums the two per-batch partials in f32 -- the added rounding is
~0.4% RMS against a 2e-2 budget). Steady state runs the PE at ~99%
and ScalarE (exp) at ~97% concurrently; both engines are at their
structural floor (PE: 512 proj + 512 attn@V + 256 dots-pair matmul
slots, all N=512 streams; ScalarE: 256 1024-wide exps).
"""

import sys

sys.path.insert(0, "/opt/trn_rl_repo")

import ml_dtypes
import numpy as np

import concourse.bass as bass
import concourse.tile as tile
from concourse import bacc, mybir
from concourse.bass_utils import run_bass_kernel_spmd
from concourse.masks import make_identity

F32 = mybir.dt.float32
BF16 = mybir.dt.bfloat16
AF = mybir.ActivationFunctionType

B, N, DIM = 4, 2048, 1024
HEADS, DH = 16, 64
INNER = HEADS * DH  # 1024
SCALE = DH ** -0.5
N_CORES = 8
RT = N // 128         # 16 row tiles
KT = DIM // 128       # 8 contraction tiles (qkv proj)
MYH = HEADS // 2      # 8 heads per core
PAIRS = MYH // 2      # 4 head pairs per core
CHS = N // 512        # 4 query chunks of 512
KO = MYH * DH // 128  # 4 contraction tiles (out proj)
VW = MYH * 65 + 63    # v_sb free width per row-tile (65-per-head + pad)
EPS = 1e-5


def _build_graph():
    nc = bacc.Bacc("TRN2", target_bir_lowering=False, debug=False,
                   num_devices=N_CORES)
    x_d = nc.dram_tensor("x", [N, DIM], BF16, kind="ExternalInput").ap()
    # per-core slice: [q 512 | k 512 | v 512] columns of w_qkv
    wqkv_d = nc.dram_tensor("wqkv", [DIM, 3 * 512], BF16,
                            kind="ExternalInput").ap()
    # per-core slice: rows hh*512:(hh+1)*512 of w_out
    wout_d = nc.dram_tensor("wout", [512, DIM], BF16,
                            kind="ExternalInput").ap()
    # bf16 partial outputs: the host sums the two per-batch partials in
    # f32; the extra ~0.4% rounding is far inside the error budget and
    # halves the output DMA (8MB -> 4MB per core)
    out_d = nc.dram_tensor("out", [N, DIM], BF16, kind="ExternalOutput").ap()

    with tile.TileContext(nc) as tc:
        _kernel_body(tc, x_d, wqkv_d, wout_d, out_d)
    nc.compile()
    return nc


def _kernel_body(tc, x_d, wqkv_d, wout_d, out_d):
    nc = tc.nc
    from contextlib import ExitStack

    with ExitStack() as outer:
        const_pool = outer.enter_context(tc.tile_pool(name="const", bufs=1))
        persist = outer.enter_context(tc.tile_pool(name="persist", bufs=1))
        # NOTE: denser PE packing (3 dots buffers + 2-t grouping) was
        # tried and REGRESSED: the sustained matmul density trips the
        # chip's activity throttle / HAM oscillations. The ~107ns
        # contraction-row-switch stalls per t are effectively free
        # power headroom. Keep 2+2+2+2 PSUM pools and per-t interleave.
        pools = {}
        pools["psm"] = outer.enter_context(
            tc.tile_pool(name="psm", bufs=2, space=bass.MemorySpace.PSUM))
        pools["psd"] = outer.enter_context(
            tc.tile_pool(name="psd", bufs=2, space=bass.MemorySpace.PSUM))

        ident = const_pool.tile([128, 128], BF16, tag="ident")
        make_identity(nc, ident[:])
        eps_t = const_pool.tile([128, 1], F32, tag="eps")
        nc.gpsimd.memset(eps_t[:], EPS)
        # preload the Sqrt and Exp activation tables with dummy ops while
        # the first x DMA is still in flight -- the first real rstd/exp
        # otherwise pays a ~1.3us ACT_TABLE_LOAD on the critical path
        warm_t = const_pool.tile([1, 1], F32, tag="actwarm")
        nc.scalar.activation(out=warm_t[:], in_=eps_t[0:1, :], func=AF.Sqrt)
        nc.scalar.activation(out=warm_t[:], in_=eps_t[0:1, :], func=AF.Exp)


        # persistent through the whole kernel
        xnt = persist.tile([128, KT, N], BF16, tag="xnt")      # [dim, kt, row]
        v_sb = persist.tile([128, RT, VW], BF16, tag="v")      # [kv,rt,h*65+d]

        # per-pair projection staging (lives through the pipeline)
        wqkp = outer.enter_context(tc.tile_pool(name="wqkp", bufs=2))
        qtp_pool = outer.enter_context(tc.tile_pool(name="qtp", bufs=2))
        ktp_pool = outer.enter_context(tc.tile_pool(name="ktp", bufs=2))
        ap_pool = outer.enter_context(tc.tile_pool(name="attn", bufs=2))

        def stage_weights(p):
            wqk = wqkp.tile([128, KT, 256], BF16, tag="wqk")
            for k in range(KT):
                nc.sync.dma_start(
                    wqk[:, k, 0:128],
                    wqkv_d[k * 128:(k + 1) * 128, p * 128:(p + 1) * 128])
                nc.sync.dma_start(
                    wqk[:, k, 128:256],
                    wqkv_d[k * 128:(k + 1) * 128,
                           512 + p * 128:512 + (p + 1) * 128])
            return wqk

        def proj_q_chunk(wqk, qt_p, ch):
            ps = pools["psm"].tile([128, 512], F32, tag="mm")
            for k in range(KT):
                nc.tensor.matmul(
                    ps[:], wqk[:, k, 0:128],
                    xnt[:, k, ch * 512:(ch + 1) * 512],
                    start=(k == 0), stop=(k == KT - 1))
            nc.vector.tensor_copy(out=qt_p[:, ch * 512:(ch + 1) * 512],
                                  in_=ps[:])

        def proj_k_chunk(wqk, kt_p, ch):
            ps = pools["psm"].tile([128, 512], F32, tag="mm")
            for k in range(KT):
                nc.tensor.matmul(
                    ps[:], wqk[:, k, 128:256],
                    xnt[:, k, ch * 512:(ch + 1) * 512],
                    start=(k == 0), stop=(k == KT - 1))
            nc.vector.tensor_copy(out=kt_p[:, ch * 512:(ch + 1) * 512],
                                  in_=ps[:])

        def alloc_pair():
            qt_p = qtp_pool.tile([128, N], BF16, tag="qt")
            kt_p = ktp_pool.tile([128, N], BF16, tag="kt")
            return qt_p, kt_p

        def dots_exp(qt_p, kt_p, at, ch, t):
            # both heads of the pair into one [128,1024] psum (2 banks),
            # K=64 row tiles 0/64 run concurrently in the PE array.
            # NOTE: a bf16-PSUM variant via is_transpose (1-bank tiles,
            # 2-t grouping + 2048-wide exps) compiles but crashes the
            # device at runtime -- TRN2 matmul output must be fp32.
            ps = pools["psd"].tile([128, 1024], F32, tag="dots")
            for hi in range(2):
                nc.tensor.matmul(
                    ps[:, hi * 512:(hi + 1) * 512],
                    kt_p[hi * DH:(hi + 1) * DH, t * 128:(t + 1) * 128],
                    qt_p[hi * DH:(hi + 1) * DH, ch * 512:(ch + 1) * 512],
                    start=True, stop=True)
            nc.scalar.activation(out=at[:, t, :, :], in_=ps[:], func=AF.Exp)

        # ---- phase 1: LayerNorm + transpose into xnt; V projection ----
        # x row-tiles 0..3 are DMA'd before any weight staging so LN and
        # the PE transposes start within ~2us of kernel entry.
        at00 = ap_pool.tile([128, RT, 2, 512], BF16, tag="at")
        at01 = ap_pool.tile([128, RT, 2, 512], BF16, tag="at")

        def attn_v_norm(po, p, ch, hi):
            # broadcast denom (PSUM row 64) to 64 partitions, then
            # fast-approx reciprocal and one fused multiply-store
            rc = sm_pool.tile([64, 512], F32, tag="rc")
            nc.vector.stream_shuffle(out=rc[0:32, :], in_=po[64:96, :],
                                     mask=[0] * 32)
            nc.vector.stream_shuffle(out=rc[32:64, :], in_=po[64:96, :],
                                     mask=[0] * 32)
            rcr = sm_pool.tile([64, 512], F32, tag="rcr")
            nc.vector.reciprocal_approx_fast(out=rcr[:], in_=rc[:])
            nc.vector.tensor_tensor(
                out=ot[hi * DH:(hi + 1) * DH,
                       p * N + ch * 512:p * N + (ch + 1) * 512],
                in0=po[0:64, :], in1=rcr[:], op=mybir.AluOpType.mult)

        def attn_v_mms(po, at, p, ch, hi, t0, t1):
            h = 2 * p + hi
            for t in range(t0, t1):
                # stationary is a 128-col view [V_h | ones | junk]
                # (FWL path); PSUM rows 65:128 are never read
                nc.tensor.matmul(
                    po[:],
                    v_sb[:, t, h * 65:h * 65 + 128],
                    at[:, t, hi, :],
                    start=(t == 0), stop=(t == RT - 1))

        with ExitStack() as ph1:
            xp = ph1.enter_context(tc.tile_pool(name="xp", bufs=4))
            xnp = ph1.enter_context(tc.tile_pool(name="xnp", bufs=3))
            stat = ph1.enter_context(tc.tile_pool(name="stat", bufs=4))
            wvp = ph1.enter_context(tc.tile_pool(name="wvp", bufs=1))
            pst = ph1.enter_context(
                tc.tile_pool(name="pst", bufs=2, space=bass.MemorySpace.PSUM))


            early_x = []
            for rt in range(4):
                x_t = xp.tile([128, DIM], BF16, tag="x")
                # split across DMA queues: one 256KB tile on a single
                # queue takes ~11us; halves land in ~6. The second half
                # goes through the idle GPSIMD sequencer so the Sync
                # queue's descriptor generation doesn't serialize.
                nc.sync.dma_start(x_t[:, 0:512],
                                  x_d[rt * 128:(rt + 1) * 128, 0:512])
                nc.gpsimd.dma_start(x_t[:, 512:1024],
                                    x_d[rt * 128:(rt + 1) * 128, 512:1024])
                early_x.append(x_t)

            # ~3.4us of dummy matmuls while the first x DMA is in
            # flight: the PE HAM clock gate needs one busy SHORT window
            # before it releases 2.4 GHz, so warm it on garbage instead
            # of on the first real transposes/projections
            pewarm = pools["psm"].tile([128, 512], F32, tag="mm")
            for _ in range(64):
                nc.tensor.matmul(pewarm[:, 0:64], ident[:], ident[:, 0:64],
                                 start=True, stop=True)

            wv_sb = wvp.tile([128, KT, 512], BF16, tag="wv")
            for k in range(KT):
                nc.sync.dma_start(wv_sb[:, k, :],
                                  wqkv_d[k * 128:(k + 1) * 128, 1024:1536])
            wqk0 = stage_weights(0)
            pair0 = alloc_pair()

            def ln_rt(rt):
                # LN chain for one row-tile -> bf16 xn tile
                if rt < 4:
                    x_t = early_x[rt]
                else:
                    x_t = xp.tile([128, DIM], BF16, tag="x")
                    nc.sync.dma_start(x_t[:, 0:512],
                                      x_d[rt * 128:(rt + 1) * 128, 0:512])
                    nc.gpsimd.dma_start(x_t[:, 512:1024],
                                        x_d[rt * 128:(rt + 1) * 128,
                                            512:1024])

                stats = stat.tile([128, 2, 6], F32, tag="bnst")
                xr = x_t[:].rearrange("p (s f) -> p s f", s=2)
                for s in range(2):
                    nc.vector.bn_stats(out=stats[:, s, :], in_=xr[:, s, :])
                mv = stat.tile([128, 2], F32, tag="bnag")
                nc.vector.bn_aggr(out=mv[:], in_=stats[:])
                rstd = stat.tile([128, 1], F32, tag="rstd")
                nc.scalar.activation(out=rstd[:], in_=mv[:, 1:2], func=AF.Sqrt,
                                     bias=eps_t[:], scale=1.0)
                nc.vector.reciprocal(out=rstd[:], in_=rstd[:])

                # NOTE: tried GPSIMD for this (idle engine) -- its SW
                # tensor_scalar runs ~30x slower than DVE (14.8us/tile).
                xn_t = xnp.tile([128, DIM], BF16, tag="xn")
                nc.vector.tensor_scalar(
                    out=xn_t[:], in0=x_t[:], scalar1=mv[:, 0:1],
                    scalar2=rstd[:], op0=mybir.AluOpType.subtract,
                    op1=mybir.AluOpType.mult)
                return xn_t

            # LN runs two row-tiles ahead of the PE (transposes never
            # wait on the serial DVE->ACT->DVE LayerNorm chain, even
            # during the DMA-queue ramp)
            ln_q = [ln_rt(0), ln_rt(1)]
            for rt in range(RT):
                xn_t = ln_q.pop(0)
                if rt + 2 < RT:
                    ln_q.append(ln_rt(rt + 2))

                # 8 transposes share one PSUM bank; one wide copy drains
                # them all (fewer DVE ops -> phase 1 is DVE-limited)
                ps = pst.tile([128, KT, 128], BF16, tag="tr")
                for k in range(KT):
                    nc.tensor.transpose(ps[:, k, :],
                                        xn_t[:, k * 128:(k + 1) * 128],
                                        ident[:])
                if rt < 3:
                    # ScalarE is idle before the pair-0 exps start
                    nc.scalar.copy(
                        out=xnt[:, :, rt * 128:(rt + 1) * 128], in_=ps[:])
                else:
                    nc.vector.tensor_copy(
                        out=xnt[:, :, rt * 128:(rt + 1) * 128], in_=ps[:])

                # pair-0 Q/K projections staggered in as rows become
                # ready, then that milestone's 4 dots/exps for chunk
                # (0,0) -- dots t only needs K-chunk t//4, so the first
                # exps start at rt 3. 4-exp blocks (not 1/rt): each
                # Sqrt<->Exp switch costs a ~1.3us ACT table reload, and
                # the 2-ahead LN pipeline absorbs the queue delay.
                if rt in (3, 7, 11, 15):
                    j = rt // 4
                    proj_k_chunk(wqk0, pair0[1], j)
                    if rt == 3:
                        proj_q_chunk(wqk0, pair0[0], 0)
                    for t in range(4 * j, 4 * j + 4):
                        dots_exp(pair0[0], pair0[1], at00, 0, t)
                elif rt in (9, 13):
                    # Q chunks off the milestone rts: chunk (0,0)'s dots
                    # only read Q chunk 0, and the milestone rts already
                    # stack K/V drains + LN stats on DVE
                    proj_q_chunk(wqk0, pair0[0], (rt - 5) // 4)
                if rt == 15:
                    proj_q_chunk(wqk0, pair0[0], 3)
                    # also start chunk (0,1): its first two dots/exps
                    # here let rt15's V-proj absorb the psd-rotation
                    # wait that otherwise stalls the phase-2 entry
                    for t in range(2):
                        dots_exp(pair0[0], pair0[1], at01, 1, t)

                # V rows for this row-tile (needs only this rt of xnt):
                # one N=512 chain covers all 8 heads
                psv = pools["psm"].tile([128, 512], F32, tag="mm")
                for k in range(KT):
                    nc.tensor.matmul(
                        psv[:],
                        xnt[:, k, rt * 128:(rt + 1) * 128],
                        wv_sb[:, k, :],
                        start=(k == 0), stop=(k == KT - 1))
                psvr = psv[:].rearrange("p (h d) -> p h d", d=DH)
                vdst = v_sb[:, rt, 0:MYH * 65].rearrange(
                    "p (h d) -> p h d", d=65)
                if rt in (3, 7, 11, 15):
                    # milestone rts run exps on ScalarE; keep the V
                    # drain off it there (DVE), ScalarE otherwise
                    nc.vector.tensor_copy(out=vdst[:, :, 0:DH], in_=psvr[:])
                else:
                    nc.scalar.copy(out=vdst[:, :, 0:DH], in_=psvr[:])
            # ones column (softmax denominator) + finite pad for the
            # 128-wide stationary reads of head 7
            nc.gpsimd.memset(
                v_sb[:, :, 0:MYH * 65].rearrange(
                    "p r (h d) -> p r h d", d=65)[:, :, :, 64:65], 1.0)
            nc.gpsimd.memset(v_sb[:, :, MYH * 65:VW], 0.0)

        # opened only now: ot/wout do not count against the phase-1
        # SBUF peak (pool space is reserved at pool open)
        otp = outer.enter_context(tc.tile_pool(name="otp", bufs=1))
        ot = otp.tile([128, PAIRS * N], BF16, tag="ot")
        wout_sb = otp.tile([128, KO, DIM], BF16, tag="wout")
        sm_pool = outer.enter_context(tc.tile_pool(name="smal", bufs=2))
        for k in range(KO):
            nc.sync.dma_start(wout_sb[:, k, :],
                              wout_d[k * 128:(k + 1) * 128, :])



        # ---- phase 2+3: chunk pipeline ----
        # 16 chunks C[i] = (pair, ch). dots+exp for C[i+1] are emitted
        # interleaved with attn@V for C[i] (whose exps finished a full
        # chunk ago) and with the next pair's Q/K projections, so the
        # in-order PE stream never waits on ScalarE.
        with ExitStack() as att:
            pso = att.enter_context(
                tc.tile_pool(name="pso", bufs=2, space=bass.MemorySpace.PSUM))

            def proj_chunk_ops(wqk, dst, wofs, ch):
                ps = pools["psm"].tile([128, 512], F32, tag="mm")
                ops = []
                for k in range(KT):
                    def mm(k=k, ps=ps):
                        nc.tensor.matmul(
                            ps[:], wqk[:, k, wofs:wofs + 128],
                            xnt[:, k, ch * 512:(ch + 1) * 512],
                            start=(k == 0), stop=(k == KT - 1))
                    ops.append(mm)

                def cp(ps=ps):
                    nc.vector.tensor_copy(
                        out=dst[:, ch * 512:(ch + 1) * 512], in_=ps[:])
                ops.append(cp)
                return ops

            def outproj_ops(m, last=False):
                # op-list for one 128-row block of the out-projection
                orow_t = sm_pool.tile([128, DIM], BF16, tag="orow")
                ops = []
                for ch in range(2):
                    ps = pools["psm"].tile([128, 512], F32, tag="mm")
                    for k in range(KO):
                        def mm(k=k, ps=ps, ch=ch):
                            nc.tensor.matmul(
                                ps[:],
                                ot[:, k * N + m * 128:
                                   k * N + (m + 1) * 128],
                                wout_sb[:, k, ch * 512:(ch + 1) * 512],
                                start=(k == 0), stop=(k == KO - 1))
                        ops.append(mm)

                    def cpdma(ps=ps, ch=ch):
                        # the exp stream is over by the time the last
                        # out-proj blocks drain; use the freed ScalarE
                        eng = nc.scalar.copy if last else (
                            lambda out, in_: nc.vector.tensor_copy(
                                out=out, in_=in_))
                        eng(out=orow_t[:, ch * 512:(ch + 1) * 512],
                            in_=ps[:])
                        if last:
                            # tail blocks: split across two queues --
                            # the final DMA's latency IS the kernel tail
                            nc.sync.dma_start(
                                out_d[m * 128:(m + 1) * 128,
                                      ch * 512:ch * 512 + 256],
                                orow_t[:, ch * 512:ch * 512 + 256])
                            nc.gpsimd.dma_start(
                                out_d[m * 128:(m + 1) * 128,
                                      ch * 512 + 256:(ch + 1) * 512],
                                orow_t[:, ch * 512 + 256:(ch + 1) * 512])
                        else:
                            nc.sync.dma_start(
                                out_d[m * 128:(m + 1) * 128,
                                      ch * 512:(ch + 1) * 512],
                                orow_t[:, ch * 512:(ch + 1) * 512])
                    ops.append(cpdma)
                return ops

            chunks = [(p, ch) for p in range(PAIRS) for ch in range(CHS)]
            wqk1 = stage_weights(1)
            pair_tiles = {0: pair0,               # p -> (qt, kt[, wqk])
                          1: alloc_pair() + (wqk1,)}
            at_tiles = {(0, 0): at00, (0, 1): at01}
            # out-proj row-block m needs ot columns m*128 for ALL pairs:
            # blocks 4ch..4ch+3 unlock after pair-3 chunk ch's norms.
            # Emitted interleaved into the following chunk's t-loop.
            outproj_pending = []

            for i in range(len(chunks)):
                ac = chunks[i]                    # attn@V chunk (exps done)
                dc = chunks[i + 1] if i + 1 < len(chunks) else None
                pending = []
                if dc is not None:
                    dp = dc[0]
                    if dc[1] == 0 and dp + 1 < PAIRS:
                        # entering pair dp: stage weights+tiles for dp+1
                        wqk_n = stage_weights(dp + 1)
                        pair_tiles[dp + 1] = alloc_pair() + (wqk_n,)
                    if dp + 1 < PAIRS:
                        nq_t, nk_t, wqk_n = pair_tiles[dp + 1]
                        proj_chs = [0, 1] if i == 0 else [dc[1]]
                        for pc in proj_chs:
                            pending += proj_chunk_ops(wqk_n, nk_t, 128, pc)
                            pending += proj_chunk_ops(wqk_n, nq_t, 0, pc)
                    dq_p, dk_p = pair_tiles[dp][0], pair_tiles[dp][1]
                    at_d = at_tiles.get(dc)
                    if at_d is None:
                        at_d = ap_pool.tile([128, RT, 2, 512], BF16,
                                            tag="at")
                        at_tiles[dc] = at_d
                at_a = at_tiles.pop(ac)
                po0 = pso.tile([128, 512], F32, tag="po")
                po1 = None
                pending += outproj_pending
                outproj_pending = []

                for t in range(RT):
                    # attn@V FIRST, dots after: the attn@V stationary
                    # (128x128, FWL) background-loads behind the
                    # preceding full-row matmul, but not behind the
                    # K=64 dots pair -- ordering av before dots avoids
                    # one ~108ns LDWEIGHTS stall per t-step.
                    # attn@V for ac: hi=0 front-loaded (t<8), hi=1 after
                    if t < 8:
                        attn_v_mms(po0, at_a, ac[0], ac[1], 0,
                                   2 * t, 2 * t + 2)
                        if t == 7:
                            attn_v_norm(po0, ac[0], ac[1], 0)
                    else:
                        if t == 8:
                            po1 = pso.tile([128, 512], F32, tag="po")
                        attn_v_mms(po1, at_a, ac[0], ac[1], 1,
                                   2 * (t - 8), 2 * (t - 8) + 2)
                        if t == RT - 1:
                            attn_v_norm(po1, ac[0], ac[1], 1)
                    if dc is not None and not (dc == (0, 1) and t < 2):
                        dots_exp(dq_p, dk_p, at_d, dc[1], t)
                    if t >= 2 and pending:
                        # drain faster when the queue is long (i=0's
                        # double proj stage, the out-proj-heavy tail
                        # chunks) so the end-of-chunk flush below never
                        # emits a long serial run
                        n_pop = 3 if (dc is None or len(pending) > 24) \
                            else 2
                        for _ in range(n_pop):
                            if pending:
                                pending.pop(0)()
                while pending:
                    pending.pop(0)()

                if ac[0] == PAIRS - 1:
                    # pair-3 chunk done: out-proj rows ac[1]*512 +
                    # 0:512 are fully normalized; queue their 4 blocks
                    for m in range(4 * ac[1], 4 * ac[1] + 4):
                        outproj_pending += outproj_ops(
                            m, last=(ac[1] == CHS - 1))

            # ---- tail: remaining out-proj blocks (last chunk's rows)
            while outproj_pending:
                outproj_pending.pop(0)()


_NC_CACHE = None


def _make_in_maps(x, wqkv_bf, wout_bf):
    in_maps = []
    for core in range(N_CORES):
        b, hh = core // 2, core % 2
        # per-core w_qkv slice: q/k/v columns of this core's 8 heads
        w_sl = np.concatenate(
            [wqkv_bf[:, hh * 512:(hh + 1) * 512],
             wqkv_bf[:, INNER + hh * 512:INNER + (hh + 1) * 512],
             wqkv_bf[:, 2 * INNER + hh * 512:2 * INNER + (hh + 1) * 512]],
            axis=1)
        in_maps.append({
            "x": np.ascontiguousarray(x[b]).astype(ml_dtypes.bfloat16),
            "wqkv": np.ascontiguousarray(w_sl),
            "wout": np.ascontiguousarray(wout_bf[hh * 512:(hh + 1) * 512, :]),
        })
    return in_maps


def kernel(x, ln_gamma, ln_beta, w_qkv, w_out, b_out):
    global _NC_CACHE
    x = np.asarray(x, dtype=np.float32)
    ln_gamma = np.asarray(ln_gamma, dtype=np.float32)
    ln_beta = np.asarray(ln_beta, dtype=np.float32)
    w_qkv = np.asarray(w_qkv, dtype=np.float32)
    w_out = np.asarray(w_out, dtype=np.float32)
    b_out = np.asarray(b_out, dtype=np.float32)

    # fold gamma + softmax scale into w_qkv (host, exact f32)
    wqkv_eff = w_qkv * ln_gamma[:, None]
    wqkv_eff = wqkv_eff.copy()
    wqkv_eff[:, :INNER] *= SCALE
    qkv_bias = ln_beta @ w_qkv
    assert not np.any(qkv_bias), "nonzero ln_beta not supported on device"
    wqkv_bf = wqkv_eff.astype(ml_dtypes.bfloat16)
    wout_bf = w_out.astype(ml_dtypes.bfloat16)

    if _NC_CACHE is None:
        _NC_CACHE = _build_graph()
    nc = _NC_CACHE

    # clear any wedged NRT state left by a previous process on the cores
    try:
        import ctypes
        import jax
        jax.devices()
        _lib = ctypes.CDLL("/opt/axon/libaxon_pjrt.so")
        if hasattr(_lib, "axon_reset"):
            _lib.axon_reset.restype = ctypes.c_int64
            _lib.axon_reset()
    except Exception:
        pass

    in_maps = _make_in_maps(x, wqkv_bf, wout_bf)
    res = run_bass_kernel_spmd(nc, in_maps, core_ids=list(range(N_CORES)))

    out = np.empty((B, N, DIM), dtype=np.float32)
    for b in range(B):
        out[b] = np.asarray(res.results[2 * b]["out"], dtype=np.float32)
        out[b] += np.asarray(res.results[2 * b + 1]["out"],
                             dtype=np.float32)
    out += b_out
    return out
